# revision 80
# baseline (speedup 1.0000x reference)
"""Trainium2 Bass kernel for nn_Encoder (VGAE-style GNN encoder).

Computation (see reference):
  deg/norms from src/dst; h = relu(ndst * segsum_dst((feat*nsrc @ W1)[src]))
  agg2 = segsum_dst((h*nsrc)[src]); mu = (agg2*ndst) @ W_mu + b_mu ; ls likewise
  z = mu + noise * exp(log_sigma)

Strategy (graph/data parallel, dst-sharded, K-grid + one-hot-spill reduce):
  - Nodes globally sorted by in-degree and striped across (core, supertile)
    so every core sees the same degree profile (the SPMD program is shared
    across cores, so all gather/matmul shapes are max-over-cores). Host
    pre-permutes feat/noise/norms into table order, un-permutes z. Fake (pad)
    nodes are spread over the last supertile of every core so each gather
    window contains a zero row for pad slots.
  - Message tables (x1, h) stored fp8e4 in 256B-stride rows (64B payload):
    each dma_gather descriptor hits the 7ns cost-model floor (vs 11.4ns for
    128B f16). 4 source windows (int16 gather index range).
  - Per (supertile, window): a K-deep grid where slot (k, p) holds the k-th
    window-w in-edge of dst slot p (pad slots fetch the window's zero row).
    The reduce is a PSUM-accumulated matmul with an fp8 identity stationary,
    batched over supertile prefixes per (window, k, bank) -- K is monotone
    along each group so prefixes are contiguous -- with no per-edge DVE
    one-hot work. Edges beyond the grid depth take a spill path with classic
    one-hot blocks (f16 stationary x fp8 moving, mixed-dtype matmul). K is
    host-optimized per (supertile, window); serpentine supertile->group
    dealing equalizes per-group rows (SBUF tile caps).
  - Software-pipelined emission: group g's epilogue is issued between group
    g+1's gathers and its reduce matmuls, so the in-order PE queue drains
    epilogues during gather waits. Per-group h/z stores; in sim mode the
    AllGathers are stood in by per-group strided local copies.
  - Epilogues: relu on ACT with nprod = nsrc*ndst (folds round-1 post-scale
    and round-2 pre-scale; b1==0); round 2 splits work across ACT (ndst
    scale-copy, exp) / PE (transpose, W_mu, W_sig) / DVE (PSUM->SBUF copy,
    z = mu + noise*exp(ls) with mu read straight from PSUM since b_mu==0).
"""

import os
import sys
import numpy as np
from contextlib import ExitStack

if "/opt/trn_rl_repo" not in sys.path:
    sys.path.insert(0, "/opt/trn_rl_repo")

import concourse.bass as bass
import concourse.mybir as mybir
import concourse.tile as tile
from concourse.bacc import Bacc
from concourse.bass_utils import run_bass_kernel_spmd

F16 = mybir.dt.float16
F32 = mybir.dt.float32
F8 = mybir.dt.float8e4
I16 = mybir.dt.int16
ALU = mybir.AluOpType
ACTF = mybir.ActivationFunctionType

ST = 128


def raw_gather(gp, out_ap, in_ap, idxs_ap, num_idxs, num_idxs_reg, elem_size,
               elem_step, single_packet=False, queue_num=0):
    """dma_gather without the elem_size_bytes%256 assert (non-transpose, DRAM
    source, 256B-aligned row stride): fetches sub-row payloads (64B fp8 /
    128B f16) from a 256B-stride table."""
    from concourse.ap_utils import ap_is_contiguous
    assert idxs_ap.dtype == mybir.dt.int16
    assert in_ap.dtype == out_ap.dtype
    assert ap_is_contiguous(in_ap.ap[1:])
    assert ap_is_contiguous(out_ap.ap[1:])
    assert ap_is_contiguous(idxs_ap.ap[1:])
    assert in_ap.ap[-1][1] == out_ap.ap[-1][1] == elem_size
    assert out_ap.ap[0][1] * out_ap.ap[1][1] == -(-num_idxs // 128) * 128
    assert in_ap.ap[0][0] == elem_step
    stride_bytes = elem_step * mybir.dt.size(in_ap.dtype)
    stride_bytes_256 = stride_bytes // 256
    assert stride_bytes % 256 == 0 and stride_bytes_256 < 256
    _in_ap = gp.lower_ap_dma(in_ap, for_custom_bir_dma=True)
    _idxs_ap = gp.lower_ap(idxs_ap)
    _out_ap = gp.lower_ap(out_ap)
    return gp.add_instruction(
        mybir.InstDMAGatherAnt(
            name=gp.bass.get_next_instruction_name(),
            ins=[*_in_ap, _idxs_ap, gp.lower_val_access(gp.to_reg(num_idxs_reg))],
            outs=[_out_ap],
            transpose=False,
            num_idxs=num_idxs,
            elem_size=elem_size,
            stride_bytes_256=stride_bytes_256,
            gen_mode=0,
            single_packet=single_packet,
            queue_num=queue_num,
            sbuf_tokens_per_rank=0,
            sbuf_free_dim_per_rank=0,
            sbuf_free_dim_pad_per_rank=0,
            sbuf_byte_offset=0,
        )
    )


def default_cfg(n, e, f, h):
    ncore = 8
    shard = -(-n // (ncore * ST)) * ST
    npad = shard * ncore
    nst = shard // ST
    nwin = 4
    win = -(-npad // nwin)  # p-major table rows per window
    assert win <= 32768, "int16 gather index range"
    sb = int(os.environ.get("KSB", "14"))
    while nst % sb:
        sb -= 1
    return dict(N=n, E=e, F=f, H=h, NCORE=ncore, SHARD=shard, NPAD=npad,
                NWIN=nwin, WIN=win, NST=nst, SB=sb, NG=nst // sb)


def _serpentine_deal(nst, ng, sb):
    """deal_to_st[d] = table supertile index (g*sb + j) of the d-th
    degree-ranked supertile: snake-deal ranks across groups so per-group
    row totals stay balanced while j stays degree-descending in each group."""
    deal_to_st = np.empty(nst, dtype=np.int64)
    d = 0
    for p in range(sb):
        order = range(ng) if p % 2 == 0 else range(ng - 1, -1, -1)
        for gi in order:
            deal_to_st[d] = gi * sb + p
            d += 1
    return deal_to_st


def build_plan(src, dst, cfg):
    """Host-side index preprocessing. Returns per-core arrays + uniform meta."""
    N, NCORE = cfg["N"], cfg["NCORE"]
    SHARD, NPAD, NWIN, WIN, NST, SB, NG = (
        cfg[k] for k in ("SHARD", "NPAD", "NWIN", "WIN", "NST", "SB", "NG"))
    src = np.asarray(src).astype(np.int64)
    dst = np.asarray(dst).astype(np.int64)

    deg_in = np.bincount(dst, minlength=NPAD)
    # global in-degree sort; stripe supertiles across cores so all cores share
    # one degree profile (SPMD program uniformity)
    order = np.argsort(-deg_in, kind="stable")
    # spread the fake (zero) nodes across the last 8 global supertiles (one
    # per core) so every gather window contains a zero row for pad slots
    nfake = NPAD - N
    if nfake:
        tailn = max(1024, -(-nfake // 8) * 8 * 2)
        tailn = min(tailn, NPAD)
        tail = order[-tailn:].copy()
        fk = tail[tail >= N]
        rl = tail[tail < N]
        newtail = np.empty_like(tail)
        taken = np.zeros(tailn, dtype=bool)
        nst_tail = min(8, tailn // ST)
        for j, f in enumerate(fk):
            o2 = (j % nst_tail) * ST + j // nst_tail
            newtail[o2] = f
            taken[o2] = True
        newtail[~taken] = rl
        order[-tailn:] = newtail
    rank = np.arange(NPAD)
    sti = rank // ST                      # global sorted supertile 0..NST*8-1
    st_core = sti % NCORE
    deal_to_st = _serpentine_deal(NST, NG, SB)
    st_in_core = deal_to_st[sti // NCORE]
    posn = st_core * SHARD + st_in_core * ST + (rank % ST)
    pos_of = np.empty(NPAD, dtype=np.int64)
    pos_of[order] = posn

    # p-major table row of a position (matches phase-1 store layout)
    def srow_of(pos):
        return (pos // SHARD) * SHARD + (pos % SHARD) % ST * NST + (pos % SHARD) // ST

    srow_all = srow_of(np.arange(NPAD))
    # overlapping gather windows: width = full int16 range, bases evenly
    # spaced so ~30% of rows fall in two windows; per-dst balancing of
    # flexible edges lowers the per-window degree maxima (grid depth K)
    WINW = 32768
    if NWIN > 1:
        SPC = (NPAD - WINW) // (NWIN - 1)
        bases = np.array([w * SPC for w in range(NWIN - 1)]
                         + [NPAD - WINW], dtype=np.int64)
    else:
        bases = np.zeros(1, dtype=np.int64)
    assert all(b + WINW <= NPAD for b in bases) and bases[-1] + WINW == NPAD
    # zero-pad target row per window: a fake position (node id >= N) in range
    fake_pos = pos_of[N:] if NPAD > N else None
    zpad = np.zeros(NWIN, dtype=np.int64)
    if fake_pos is not None and len(fake_pos):
        fr = np.sort(srow_of(fake_pos))
        for w in range(NWIN):
            cand = fr[(fr >= bases[w]) & (fr < bases[w] + WINW)]
            assert len(cand), f"no fake row in window {w}"
            zpad[w] = cand[0]
    groups = [[g * SB + j for j in range(SB)] for g in range(NG)]

    dstp = pos_of[dst]
    srcp = pos_of[src]
    srow = srow_of(srcp)
    core = dstp // SHARD
    stl = (dstp % SHARD) // ST
    slot = dstp % ST

    # per-edge window assignment: rows in two windows' ranges are balanced
    # per dst to flatten deg_w(d) (waterfill over adjacent window pairs)
    if NWIN > 1:
        wminE = np.maximum(0, -(-(srow - (WINW - 1)) // SPC)).astype(np.int64)
        wmaxE = np.minimum(NWIN - 1, srow // SPC).astype(np.int64)
    else:
        wminE = wmaxE = np.zeros(len(srow), dtype=np.int64)
    flex = wmaxE > wminE
    f = np.zeros((NPAD, NWIN), dtype=np.int32)
    np.add.at(f, (dstp[~flex], wminE[~flex]), 1)
    npair = np.zeros((NPAD, max(NWIN - 1, 1)), dtype=np.int32)
    if flex.any():
        np.add.at(npair, (dstp[flex], wminE[flex]), 1)
    x = np.zeros_like(npair)
    c = f.copy()
    c[:, :NWIN - 1] += npair[:, :NWIN - 1] if NWIN > 1 else 0
    for _ in range(16):
        moved = False
        for i in range(NWIN - 1):
            mv = (c[:, i] > c[:, i + 1] + 1) & (x[:, i] < npair[:, i])
            if mv.any():
                x[mv, i] += 1
                c[mv, i] -= 1
                c[mv, i + 1] += 1
                moved = True
            bk = (c[:, i + 1] > c[:, i] + 1) & (x[:, i] > 0)
            if bk.any():
                x[bk, i] -= 1
                c[bk, i] += 1
                c[bk, i + 1] -= 1
                moved = True
        if not moved:
            break
    wofe = wminE.copy()
    if flex.any():
        idxf = np.nonzero(flex)[0]
        keyf = dstp[idxf] * (NWIN - 1) + wminE[idxf]
        of = np.argsort(keyf, kind="stable")
        kf = keyf[of]
        rk = np.arange(len(kf)) - np.searchsorted(kf, kf, side="left")
        sel = idxf[of]
        upper = rk < x[dstp[sel], wminE[sel]]
        wofe[sel] = wminE[sel] + upper

    # per-core per-(st,w) degree per slot
    deg = np.zeros((NCORE, NST, NWIN, ST), dtype=np.int32)
    key = ((core * NST + stl) * NWIN + wofe) * ST + slot
    cnts = np.bincount(key, minlength=NCORE * NST * NWIN * ST)
    deg = cnts.reshape(NCORE, NST, NWIN, ST)

    # K optimization per (st, w), SPMD-uniform across cores:
    # cost(K) = 128*K + 128*max_c ceil(spill_c(K)/128)
    degM = deg  # [C, NST, NWIN, ST]
    maxdeg = int(degM.max())
    # a spill block costs more than a grid row (DVE one-hot build + per-block
    # Ldweights/matmul issue vs one batched identity matmul)
    spill_w = float(os.environ.get("KSPW", "1.2"))
    K = np.zeros((NST, NWIN), dtype=np.int32)
    SPB = np.zeros((NST, NWIN), dtype=np.int32)
    for s in range(NST):
        for w in range(NWIN):
            d = degM[:, s, w, :]  # [C, ST]
            best, bestk, bestspb = None, 0, 0
            for k in range(0, min(maxdeg, int(d.max())) + 1):
                spill = np.maximum(d - k, 0).sum(axis=1).max()
                spb = -(-int(spill) // ST)
                c = ST * k + spill_w * ST * spb
                if best is None or c < best:
                    best, bestk, bestspb = c, k, spb
            K[s, w], SPB[s, w] = bestk, bestspb
    # monotone non-increasing K along each group's st order (prefix matmuls)
    for g in range(NG):
        sts = groups[g]
        for w in range(NWIN):
            for j in range(SB - 2, -1, -1):
                K[sts[j], w] = max(K[sts[j], w], K[sts[j + 1], w])
    # recompute spill block counts at the final K
    for s in range(NST):
        for w in range(NWIN):
            spill = np.maximum(degM[:, s, w, :] - K[s, w], 0).sum(axis=1).max()
            SPB[s, w] = -(-int(spill) // ST)

    # uniform row layout per (g, w): grid rows k-major, then spill rows
    rows_gw = np.zeros((NG, NWIN), dtype=np.int64)
    grid_layout = {}   # (g,w) -> list of (k, n_k, row_start)
    spill_layout = {}  # (g,w) -> list of (j, st, row_start, nblocks)
    for g in range(NG):
        sts = groups[g]
        for w in range(NWIN):
            r = 0
            gl = []
            kmax = int(K[sts[0], w])
            for k in range(kmax):
                nk = int(sum(1 for s in sts if K[s, w] > k))
                gl.append((k, nk, r))
                r += nk
            sl = []
            bloc = 0
            for j, s in enumerate(sts):
                nb = int(SPB[s, w])
                if nb:
                    sl.append((j, s, r, nb, bloc))
                    r += nb
                    bloc += nb
            grid_layout[(g, w)] = gl
            spill_layout[(g, w)] = sl
            rows_gw[g, w] = r
    ROWS_CAP = int(rows_gw.max())
    # spill block base offsets in dstloc, ordered (g, w); NBMAX = per-(g,w) max
    spb_off = {}
    acc = 0
    NBMAX = 1
    for g in range(NG):
        for w in range(NWIN):
            spb_off[(g, w)] = acc
            nb_gw = sum(nb for (_, _, _, nb, _) in spill_layout[(g, w)])
            NBMAX = max(NBMAX, nb_gw)
            acc += nb_gw
    NSPB = acc
    # eidx col offsets per (g, w)
    col_off = {}
    acc = 0
    for g in range(NG):
        for w in range(NWIN):
            col_off[(g, w)] = acc
            acc += int(rows_gw[g, w]) * (ST // 16)
    TOTCOLS = acc

    # per-core edge placement
    plans = []
    for c in range(NCORE):
        sel = core == c
        e_st, e_w, e_slot = stl[sel], wofe[sel], slot[sel]
        e_srow = srow[sel]
        o = np.lexsort((e_srow, e_slot, e_w, e_st))
        e_st, e_w, e_slot, e_srow = e_st[o], e_w[o], e_slot[o], e_srow[o]
        # rank of edge within its (st, w, slot) list
        key2 = (e_st * NWIN + e_w) * ST + e_slot
        # edges sorted by key2 groups (lexsort above ensures grouping)
        o2 = np.argsort(key2, kind="stable")
        k2s = key2[o2]
        within = np.arange(len(k2s)) - np.searchsorted(k2s, k2s, side="left")
        e_k = np.empty(len(k2s), dtype=np.int64)
        e_k[o2] = within

        eidx = np.zeros((128, TOTCOLS), dtype=np.int16)
        dloc = np.full((128, max(NSPB, 1)), 300.0, dtype=np.float16)
        st_j = {}
        for g in range(NG):
            for j, s in enumerate(groups[g]):
                st_j[s] = (g, j)
        # grid placement
        for g in range(NG):
            sts = groups[g]
            jmap = -np.ones(NST, dtype=np.int64)
            for j, s in enumerate(sts):
                jmap[s] = j
            for w in range(NWIN):
                r_gw = int(rows_gw[g, w])
                if r_gw == 0:
                    continue
                idx_flat = np.full(r_gw * ST, zpad[w] - bases[w], dtype=np.int64)
                # grid rows
                gsel = (np.isin(e_st, sts) & (e_w == w)
                        & (e_k < K[e_st, w]))
                gs = np.nonzero(gsel)[0]
                if len(gs):
                    kk = e_k[gs]
                    ss = e_st[gs]
                    jj = jmap[ss]
                    # row of (k, st j): row_start(k) + position of j among
                    # sts with K> k (prefix since K monotone in j)
                    gl = grid_layout[(g, w)]
                    rstart = np.zeros(int(K[sts[0], w]) + 1, dtype=np.int64)
                    for (k, nk, rs) in gl:
                        rstart[k] = rs
                    rows = rstart[kk] + jj
                    idx_flat[rows * ST + e_slot[gs]] = e_srow[gs] - bases[w]
                # spill rows
                for (j, s, rs, nb, bloc) in spill_layout[(g, w)]:
                    ssel = np.nonzero((e_st == s) & (e_w == w)
                                      & (e_k >= K[s, w]))[0]
                    assert len(ssel) <= nb * ST
                    boff = spb_off[(g, w)] + bloc
                    for i, ei in enumerate(ssel):
                        b, p = divmod(i, ST)
                        idx_flat[(rs + b) * ST + p] = e_srow[ei] - bases[w]
                        dloc[p, boff + b] = np.float16(e_slot[ei])
                ncols = r_gw * (ST // 16)
                wrapped = idx_flat.astype(np.int16).reshape(ncols, 16).T
                c0 = col_off[(g, w)]
                eidx[:, c0:c0 + ncols] = np.tile(wrapped, (8, 1))
        plans.append(dict(eidx=eidx, dstloc=dloc, pos_of=pos_of))

    slots_round = int(rows_gw.sum()) * ST
    meta = dict(K=K, SPB=SPB, groups=groups, rows_gw=rows_gw,
                grid_layout=grid_layout, spill_layout=spill_layout,
                spb_off=spb_off, col_off=col_off, TOTCOLS=TOTCOLS,
                NSPB=max(NSPB, 1), NBMAX=NBMAX, ROWS_CAP=ROWS_CAP, zpad=zpad,
                bases=bases, WINW=WINW,
                slots_round=slots_round, pos_of=pos_of)
    return plans, meta


def build_program(cfg, meta, sim_mode=False):
    NCORE, SHARD, NPAD = cfg["NCORE"], cfg["SHARD"], cfg["NPAD"]
    NWIN, WIN, NST, SB, NG, F, H = (cfg[k] for k in
                                    ("NWIN", "WIN", "NST", "SB", "NG", "F", "H"))
    groups = meta["groups"]
    rows_gw = meta["rows_gw"]
    grid_layout = meta["grid_layout"]
    spill_layout = meta["spill_layout"]
    spb_off = meta["spb_off"]
    col_off = meta["col_off"]
    TOTCOLS, NSPB, ROWS_CAP = meta["TOTCOLS"], meta["NSPB"], meta["ROWS_CAP"]
    NBMAX = meta["NBMAX"]
    BASES, WINW = meta["bases"], meta["WINW"]

    TDT = F8 if os.environ.get("KDT", "f8") == "f8" else F16
    TB = H                               # payload elements per table row
    TROW = 256 if TDT == F8 else 128     # stored elements per 256B row

    nc = Bacc(trn_type="TRN2", num_devices=NCORE)

    feat_tt = nc.dram_tensor("feat_tt", [F, SHARD], TDT, kind="ExternalInput")
    nsrc = nc.dram_tensor("nsrc", [128, NST], F32, kind="ExternalInput")
    nprod = nc.dram_tensor("nprod", [128, NST], F32, kind="ExternalInput")
    ndst = nc.dram_tensor("ndst", [128, NST], F32, kind="ExternalInput")
    w1_16 = nc.dram_tensor("w1_16", [F, H], F16, kind="ExternalInput")
    wmu_16 = nc.dram_tensor("wmu_16", [H, H], F16, kind="ExternalInput")
    wsig_16 = nc.dram_tensor("wsig_16", [H, H], F16, kind="ExternalInput")
    b1_rep = nc.dram_tensor("b1_rep", [128, H], F32, kind="ExternalInput")
    bmu_col = nc.dram_tensor("bmu_col", [H, 1], F32, kind="ExternalInput")
    bsig_col = nc.dram_tensor("bsig_col", [H, 1], F32, kind="ExternalInput")
    eye16_d = nc.dram_tensor("eye16", [128, 128], F16, kind="ExternalInput")
    ident_d = nc.dram_tensor("ident_t", [128, 128], TDT, kind="ExternalInput")
    iota_rep_d = nc.dram_tensor("iota_rep", [128, 128, NBMAX], F16,
                                kind="ExternalInput")
    eidx_d = nc.dram_tensor("eidx", [128, TOTCOLS], I16, kind="ExternalInput")
    dstloc_d = nc.dram_tensor("dstloc", [128, NSPB], F16, kind="ExternalInput")
    noise_t = nc.dram_tensor("noise_t", [H, SHARD], F16, kind="ExternalInput")
    z_out = nc.dram_tensor("z_out", [H, SHARD], F16, kind="ExternalOutput")
    dbg = bool(int(os.environ.get("KDBG", "0")))
    if dbg:
        x1_dbg = nc.dram_tensor("x1_dbg", [128, NST, TROW], TDT,
                                kind="ExternalOutput")
        h_dbg = nc.dram_tensor("h_dbg", [128, NST, TROW], TDT,
                               kind="ExternalOutput")

    x1_shard = nc.dram_tensor("x1_shard", [128, NST, TROW], TDT, kind="Internal")
    h_shard = nc.dram_tensor("h_shard", [128, NST, TROW], TDT, kind="Internal")
    x1_table = nc.dram_tensor("x1_table", [NPAD, TROW], TDT, kind="Internal",
                              addr_space="Shared")
    h_table = nc.dram_tensor("h_table", [NPAD, TROW], TDT, kind="Internal",
                             addr_space="Shared")
    cgroups = [list(range(NCORE))]

    spill_oh_dt = F16 if os.environ.get("KMIX", "1") == "1" else TDT

    with tile.TileContext(nc) as tc, ExitStack() as ctx:
        consts = ctx.enter_context(tc.tile_pool(name="consts", bufs=1))

        def cload(dram, shape, dtype, tag):
            t = consts.tile(shape, dtype, tag=tag)
            nc.sync.dma_start(t[:], dram[:])
            return t

        w1_sb = cload(w1_16, [F, H], F16, "w1")
        nsrc_sb = cload(nsrc, [128, NST], F32, "nsrc")
        # eidx loads in per-group slices interleaved with phase-1 emission:
        # the first gather then only waits on slice 0 (subtile deps), not the
        # whole ~10us table load
        eidx_sb = consts.tile([128, TOTCOLS], I16, tag="eidx")
        eidx_bnd = [col_off[(g, 0)] for g in range(NG)] + [TOTCOLS]

        # ---------------- phase 1: x1 = (feat*nsrc) @ W1 on own shard -------
        PG = int(os.environ.get("KPG", "49"))  # physical supertiles per feat tile
        with tc.tile_pool(name="p1", bufs=3) as p1, \
             tc.tile_pool(name="p1ps", bufs=8, space="PSUM") as p1ps:
            QB = 7  # supertiles per PSUM tile / fp8-conversion op (nsrc is
            #         folded into feat on the host, so no per-st scalar)
            for g0 in range(NST // PG):
                ftg = p1.tile([F, PG * 128], TDT, tag="ftg")
                nc.sync.dma_start(ftg[:],
                                  feat_tt[:, g0 * PG * 128:(g0 + 1) * PG * 128])
                if g0 < NG and eidx_bnd[g0] < eidx_bnd[g0 + 1]:
                    nc.sync.dma_start(
                        eidx_sb[:, eidx_bnd[g0]:eidx_bnd[g0 + 1]],
                        eidx_d[:, eidx_bnd[g0]:eidx_bnd[g0 + 1]])
                xg = p1.tile([128, PG, H], TDT, tag="xg")
                for q in range(PG // QB):
                    x1p = p1ps.tile([128, QB * H], F32, tag="x1p")
                    for si in range(QB):
                        c = q * QB + si
                        nc.tensor.matmul(
                            x1p[:, si * H:(si + 1) * H],
                            ftg[:, c * 128:(c + 1) * 128],
                            w1_sb[:], start=True, stop=True)
                    nc.vector.tensor_scalar(
                        xg[:, q * QB:(q + 1) * QB, :], x1p[:], 1.0, None,
                        ALU.mult)
                nc.sync.dma_start(
                    x1_shard[:, g0 * PG:(g0 + 1) * PG, 0:H], xg[:])
                if sim_mode:
                    # collective stand-in, pipelined per group
                    nc.sync.dma_start(
                        x1_table[0:SHARD, :]
                        .rearrange("(p s) e -> p s e", p=128)
                        [:, g0 * PG:(g0 + 1) * PG, :],
                        x1_shard[:, g0 * PG:(g0 + 1) * PG, :])
            for g0 in range(NST // PG, NG):
                if eidx_bnd[g0] < eidx_bnd[g0 + 1]:
                    nc.sync.dma_start(
                        eidx_sb[:, eidx_bnd[g0]:eidx_bnd[g0 + 1]],
                        eidx_d[:, eidx_bnd[g0]:eidx_bnd[g0 + 1]])

        # round-only consts: loaded after phase-1 emission
        wmu_sb = cload(wmu_16, [H, H], F16, "wmu")
        wsig_sb = cload(wsig_16, [H, H], F16, "wsig")
        ndst_sb = cload(ndst, [128, NST], F32, "ndst")
        nprod_sb = cload(nprod, [128, NST], F32, "nprod")
        b1_sb = cload(b1_rep, [128, H], F32, "b1")
        bmu_sb = cload(bmu_col, [H, 1], F32, "bmu")
        bsig_sb = cload(bsig_col, [H, 1], F32, "bsig")
        eye16 = cload(eye16_d, [128, 128], F16, "eye16")
        ident = cload(ident_d, [128, 128], TDT, "ident")
        iota_rep = cload(iota_rep_d, [128, 128, NBMAX], F16, "iota_rep")
        dstloc_sb = cload(dstloc_d, [128, NSPB], F16, "dstloc")

        if dbg:
            nc.sync.dma_start(x1_dbg[:], x1_shard[:, :, :])
        if not sim_mode:
            nc.gpsimd.collective_compute("AllGather", ALU.bypass, cgroups,
                                         ins=[x1_shard[:]], outs=[x1_table[:]])

        HB = min(SB, 7)  # supertiles per PSUM bank

        # ---------------- message-passing round ----------------------------
        def mp_round(table, epilogue, pre_round, post_group, rtag):
            with tc.tile_pool(name=f"msgs{rtag}", bufs=3) as msgs, \
                 tc.tile_pool(name=f"ohp{rtag}", bufs=2) as ohp, \
                 tc.tile_pool(name=f"aggps{rtag}", bufs=2, space="PSUM") as aggps, \
                 tc.tile_pool(name=f"epi{rtag}", bufs=4) as epi, \
                 tc.tile_pool(name=f"episb{rtag}", bufs=4) as episb, \
                 tc.tile_pool(name=f"stg{rtag}", bufs=1) as stg, \
                 tc.tile_pool(name=f"epips{rtag}", bufs=2, space="PSUM") as epips, \
                 tc.tile_pool(name=f"epips2{rtag}", bufs=2, space="PSUM") as epips2:
                rctx = pre_round(stg)

                def run_epilogue(g, banks, only_hb=None, do_post=True):
                    for j, s in enumerate(groups[g]):
                        if only_hb is not None and j // HB != only_hb:
                            continue
                        agg = banks[j // HB][:, (j % HB) * H:(j % HB + 1) * H]
                        epilogue(g, j, s, agg, rctx, epi, episb, epips, epips2)
                    if do_post:
                        post_group(g, rctx)

                pending = None
                for g in range(NG):
                    sts = groups[g]
                    # emit window 0 last (its table rows overlap the own-shard
                    # copy; avoids Pool head-of-line blocking at the boundary)
                    mt = {}
                    for w in (1, 2, 3, 0):
                        r_gw = int(rows_gw[g, w])
                        if r_gw == 0:
                            continue
                        m = msgs.tile([128, ROWS_CAP, TB], TDT, tag=f"m{w}")
                        raw_gather(
                            nc.gpsimd, m[:, 0:r_gw, :],
                            table[int(BASES[w]):int(BASES[w]) + WINW, 0:TB],
                            eidx_sb[:, col_off[(g, w)]:
                                    col_off[(g, w)] + r_gw * (ST // 16)],
                            num_idxs=r_gw * ST, num_idxs_reg=r_gw * ST,
                            elem_size=TB, elem_step=TROW)
                        mt[w] = m
                    # previous group's epilogue issues while this group's
                    # gathers are in flight (PE queue is in-order: emitting it
                    # before the reduce matmuls keeps PE busy during the wait)
                    if pending is not None:
                        run_epilogue(*pending)
                        pending = None
                    # spill one-hots (DVE; independent of gathers)
                    ohs = {}
                    for w in range(NWIN):
                        nb = sum(n for (_, _, _, n, _) in spill_layout[(g, w)])
                        if nb == 0:
                            continue
                        b0 = spb_off[(g, w)]
                        oh = ohp.tile([128, 128, NBMAX], spill_oh_dt,
                                      tag=f"oh{w}")
                        nc.vector.tensor_tensor(
                            oh[:, :, 0:nb], iota_rep[:, :, 0:nb],
                            dstloc_sb[:, None, b0:b0 + nb]
                            .broadcast_to([128, 128, nb]),
                            ALU.is_equal)
                        ohs[w] = oh
                    # PSUM banks: HB supertiles each
                    nbank = -(-SB // HB)
                    banks = []
                    for hb in range(nbank):
                        nsi = min(HB, SB - hb * HB)
                        ab = aggps.tile([128, nsi * H], F32, tag=f"ab{hb}")
                        nc.vector.memset(ab[:], 0.0)
                        banks.append(ab)
                    # matmul op list, bank-major so bank 0's epilogues can
                    # start before bank 1's reduce finishes; last op per bank
                    # carries the stop flag
                    ops = []
                    for hb in range(nbank):
                        for w in range(NWIN):
                            for (k, nk, rs) in grid_layout[(g, w)]:
                                lo, hi = hb * HB, min((hb + 1) * HB, nk)
                                if lo < nk:
                                    ops.append(("g", w, hb, lo, hi, rs))
                            for (j, s, rs, nb, bloc) in spill_layout[(g, w)]:
                                if j // HB != hb:
                                    continue
                                for b in range(nb):
                                    ops.append(("s", w, hb, j, rs + b,
                                                bloc + b))
                    last_of = {}
                    for i, op in enumerate(ops):
                        last_of[op[2]] = i
                    last_group = (g == NG - 1)
                    for i, op in enumerate(ops):
                        stop = (last_of[op[2]] == i)
                        if op[0] == "g":
                            _, w, hb, lo, hi, rs = op
                            nc.tensor.matmul(
                                banks[hb][:, 0:(hi - lo) * H], ident[:],
                                mt[w][:, rs + lo:rs + hi, 0:H],
                                start=False, stop=stop)
                        else:
                            _, w, hb, j, row, bi = op
                            jj = j % HB
                            nc.tensor.matmul(
                                banks[hb][:, jj * H:(jj + 1) * H],
                                ohs[w][:, :, bi],
                                mt[w][:, row, 0:H],
                                start=False, stop=stop)
                        # last group: issue each bank's epilogue as soon as
                        # that bank's accumulation stops, so the final
                        # epilogue overlaps the remaining banks' matmuls
                        if last_group and stop:
                            run_epilogue(g, banks, only_hb=op[2],
                                         do_post=False)
                    if last_group:
                        post_group(g, rctx)
                        pending = None
                    else:
                        pending = (g, banks)
                if pending is not None:
                    run_epilogue(*pending)

        # ---------------- round 1: h ----------------------------------------
        def pre_r1(stg):
            hg = stg.tile([128, NST, H], TDT, tag="hg")
            return dict(hg=hg)

        def post_g1(g, rctx):
            nc.sync.dma_start(h_shard[:, g * SB:(g + 1) * SB, 0:H],
                              rctx["hg"][:, g * SB:(g + 1) * SB, :])
            if sim_mode:
                nc.sync.dma_start(
                    h_table[0:SHARD, :]
                    .rearrange("(p s) e -> p s e", p=128)
                    [:, g * SB:(g + 1) * SB, :],
                    h_shard[:, g * SB:(g + 1) * SB, :])

        def epi_round1(g, j, s, agg, rctx, epi, episb, epips, epips2):
            if cfg.get("B1Z", True):
                nc.scalar.activation(rctx["hg"][:, s, :], agg, ACTF.Relu,
                                     scale=nprod_sb[:, s:s + 1])
            else:
                hp = epi.tile([128, H], F32, tag="hp")
                nc.vector.scalar_tensor_tensor(hp[:], agg, ndst_sb[:, s:s + 1],
                                               b1_sb[:], ALU.mult, ALU.add)
                nc.scalar.activation(rctx["hg"][:, s, :], hp[:], ACTF.Relu,
                                     scale=nsrc_sb[:, s:s + 1])

        # ---------------- round 2: z -----------------------------------------
        def pre_r2(stg):
            zg = stg.tile([H, SHARD], F16, tag="zg")
            ng = stg.tile([H, SHARD], F16, tag="ng")
            nc.sync.dma_start(ng[:], noise_t[:])
            return dict(zg=zg, ng=ng)

        def post_g2(g, rctx):
            nc.sync.dma_start(z_out[:, g * SB * 128:(g + 1) * SB * 128],
                              rctx["zg"][:, g * SB * 128:(g + 1) * SB * 128])

        bz = cfg.get("BZ", True)  # b_mu == b_sig == 0 fast path

        def epi_round2(g, j, s, agg, rctx, epi, episb, epips, epips2):
            a2s = epi.tile([128, H], F16, tag="a2s")
            nc.scalar.activation(a2s[:], agg, ACTF.Identity,
                                 scale=ndst_sb[:, s:s + 1])
            a2tp = epips.tile([H, 128], F16, tag="a2tp")
            nc.tensor.matmul(a2tp[:], a2s[:], eye16[:], is_transpose=True)
            a2t = epi.tile([H, 128], F16, tag="a2t")
            nc.vector.tensor_scalar(a2t[:], a2tp[:], 1.0, None, ALU.mult)
            musg = epips2.tile([H, 2, 128], F32, tag="musg")
            nc.tensor.matmul(musg[:, 0, :], wmu_sb[:], a2t[:], start=True,
                             stop=True)
            nc.tensor.matmul(musg[:, 1, :], wsig_sb[:], a2t[:], start=True,
                             stop=True)
            es = episb.tile([H, 128], F32, tag="es")
            if bz:
                nc.scalar.activation(es[:], musg[:, 1, :], ACTF.Exp)
            else:
                nc.scalar.activation(es[:], musg[:, 1, :], ACTF.Exp,
                                     bias=bsig_sb[:])
            nz = episb.tile([H, 128], F32, tag="nz")
            nc.vector.scalar_tensor_tensor(
                nz[:], rctx["ng"][:, s * 128:(s + 1) * 128], 1.0, es[:],
                ALU.mult, ALU.mult)
            if bz:
                nc.vector.scalar_tensor_tensor(
                    rctx["zg"][:, s * 128:(s + 1) * 128], musg[:, 0, :], 0.0,
                    nz[:], ALU.add, ALU.add)
            else:
                mub = episb.tile([H, 128], F32, tag="mub")
                nc.scalar.activation(mub[:], musg[:, 0, :], ACTF.Identity,
                                     bias=bmu_sb[:])
                nc.vector.scalar_tensor_tensor(
                    rctx["zg"][:, s * 128:(s + 1) * 128], mub[:], 0.0, nz[:],
                    ALU.add, ALU.add)

        kphase = int(os.environ.get("KPHASE", "4"))
        if kphase >= 2:
            mp_round(x1_table, epi_round1, pre_r1, post_g1, "a")
        if kphase >= 3:
            if dbg:
                nc.sync.dma_start(h_dbg[:], h_shard[:, :, :])
            if not sim_mode:
                nc.gpsimd.collective_compute("AllGather", ALU.bypass, cgroups,
                                             ins=[h_shard[:]],
                                             outs=[h_table[:]])
        if kphase >= 4:
            mp_round(h_table, epi_round2, pre_r2, post_g2, "b")

    nc.finalize()
    return nc


def host_inputs(feat, src, dst, noise, W1, b1, W_mu, b_mu, W_sig, b_sig,
                cfg, plans, meta):
    N, NCORE, SHARD, NPAD = (cfg[k] for k in ("N", "NCORE", "SHARD", "NPAD"))
    NST, F, H, NWIN = cfg["NST"], cfg["F"], cfg["H"], cfg["NWIN"]
    NSPB = meta["NSPB"]
    pos_of = meta["pos_of"]
    feat = np.asarray(feat, dtype=np.float32)
    noise = np.asarray(noise, dtype=np.float32)
    src = np.asarray(src); dst = np.asarray(dst)

    deg_out = np.bincount(src, minlength=NPAD).astype(np.float32)
    deg_in = np.bincount(dst, minlength=NPAD).astype(np.float32)
    norm_src = np.maximum(deg_out, 1.0) ** -0.5
    norm_dst = np.maximum(deg_in, 1.0) ** -0.5

    inv = np.empty(NPAD, dtype=np.int64)
    inv[pos_of] = np.arange(NPAD)          # node at each position

    featp = np.zeros((NPAD, F), dtype=np.float32)
    featp[pos_of[:N]] = feat
    noisep = np.zeros((NPAD, H), dtype=np.float32)
    noisep[pos_of[:N]] = noise
    ns_p = norm_src[inv]
    nd_p = norm_dst[inv]

    TDT8 = os.environ.get("KDT", "f8") == "f8"
    eye16 = np.eye(128, dtype=np.float16)
    if TDT8:
        import ml_dtypes
        tdt_np = ml_dtypes.float8_e4m3fn
        ident = np.eye(128).astype(tdt_np)
    else:
        tdt_np = np.float16
        ident = np.eye(128, dtype=np.float16)
    iota_rep = np.tile(np.arange(128, dtype=np.float16)[None, :, None],
                       (128, 1, meta["NBMAX"]))
    shared = dict(
        w1_16=np.asarray(W1, dtype=np.float16),
        wmu_16=np.asarray(W_mu, dtype=np.float16),
        wsig_16=np.asarray(W_sig, dtype=np.float16),
        b1_rep=np.tile(np.asarray(b1, dtype=np.float32)[None, :], (128, 1)),
        bmu_col=np.asarray(b_mu, dtype=np.float32).reshape(H, 1),
        bsig_col=np.asarray(b_sig, dtype=np.float32).reshape(H, 1),
        eye16=eye16, ident_t=ident, iota_rep=iota_rep,
    )
    in_maps = []
    for c in range(NCORE):
        lo, hi = c * SHARD, (c + 1) * SHARD
        m = dict(shared)
        m["feat_tt"] = (featp[lo:hi]
                        * ns_p[lo:hi, None]).T.astype(tdt_np).copy()
        m["nsrc"] = ns_p[lo:hi].reshape(NST, 128).T.copy()
        m["nprod"] = (ns_p * nd_p)[lo:hi].reshape(NST, 128).T.copy()
        m["ndst"] = nd_p[lo:hi].reshape(NST, 128).T.copy()
        m["noise_t"] = noisep[lo:hi].T.astype(np.float16).copy()
        m["eidx"] = plans[c]["eidx"]
        m["dstloc"] = plans[c]["dstloc"].astype(np.float16)
        in_maps.append(m)
    return in_maps


def run(feat, src, dst, noise, W1, b1, W_mu, b_mu, W_sig, b_sig,
        cfg=None, **spmd_kwargs):
    if cfg is None:
        cfg = default_cfg(feat.shape[0], src.shape[0], feat.shape[1],
                          W1.shape[1])
    cfg["B1Z"] = bool(np.all(np.asarray(b1) == 0.0))
    cfg["BZ"] = bool(np.all(np.asarray(b_mu) == 0.0)
                     and np.all(np.asarray(b_sig) == 0.0))
    plans, meta = build_plan(src, dst, cfg)
    nc = build_program(cfg, meta)
    in_maps = host_inputs(feat, src, dst, noise, W1, b1, W_mu, b_mu,
                          W_sig, b_sig, cfg, plans, meta)
    import time as _time
    last_exc = None
    for attempt in range(3):
        try:
            res = run_bass_kernel_spmd(nc, in_maps,
                                       core_ids=list(range(cfg["NCORE"])),
                                       **spmd_kwargs)
            break
        except Exception as e:
            last_exc = e
            _time.sleep(10.0)
    else:
        raise last_exc
    zs = [r["z_out"].T for r in res.results]          # [SHARD, H] each
    z_pos = np.concatenate(zs, axis=0)                # position-major
    z = z_pos[meta["pos_of"][:cfg["N"]]]
    return z.astype(np.float32), res


def kernel(feat, src, dst, noise, W1, b1, W_mu, b_mu, W_sig, b_sig):
    z, _ = run(feat, src, dst, noise, W1, b1, W_mu, b_mu, W_sig, b_sig)
    return z
